# revision 1
# baseline (speedup 1.0000x reference)
"""Trainium2 Bass kernel for cosine-similarity hint attention.

Computation (per batch b):
  sp = state_emb @ Ws.T + bs                  (B, A)
  hp = hints_emb @ Wh.T + bh                  (B, N, A)
  scores = <sp, hp> / (max(|sp|,eps) * max(|hp|,eps))
  attn = softmax(scores, axis=N)
  out = attn @ hints_emb                      (B, HD)

Distribution: data-parallel over batch, B=512 -> 64 per core on 8 cores.
Weights replicated. No collectives.

Device-side algorithm (per core, pipelined over 16 half-groups of 4
batches = 8 row-tiles of 128 hint-rows each):
  - hints arrive in two host-prepared layouts: natural [r, h] tiles in
    bf16 (for the final weighted sum, contraction over rows) and
    transposed [h, r] tiles in fp8-e4m3 (for the hint projection,
    contraction over h). Host-side work is layout/precision only.
  - hint projection z' = X @ (S*Wh.T) runs on TensorE with an augmented
    fp8 moving operand [S*Wh.T | S_q*q_b | S^2*wb] where q_b = sp_b @ Wh
    and wb = Wh.T @ bh; scales dodge fp8 subnormals and are divided out
    for free downstream. One pass yields z (256 cols), zq = <sp_b, z_r>
    and zw = <z_r, bh>.
  - |hp_r|^2 = sum((z'/S)^2) + (2/S^2)*zw' + |bh|^2; the squares run on
    ScalarE Square(scale=1/S)+accumulate (3 of every 4 tiles) and
    VectorE mult+reduce (the trailing tile of each 4), keeping both
    engines under the TensorE/DMA pace.
  - 1/max(norm, eps) = exp(-0.5*ln(max(norm^2, eps^2))) so every
    activation (Square/Ln/Exp) lives in one ACT table (no reloads).
  - softmax needs no max-subtraction (cosine scores are in [-1,1]);
    exp(scores) is scattered straight into a block-diagonal bf16
    stationary (stride-17 writes) so all 8 batches of a group
    accumulate in one [8, 512] PSUM tile over 16 weighted-sum matmuls;
    the normalizer (ones-vector matmul + pair-reduce) divides at the
    output copy.
  - emission is software-pipelined: each half-group's PE-heavy tail
    (weighted sum + normalizer) is deferred 3 halves so TensorE never
    head-of-line blocks on the DVE/ACT epilogue chain.
"""

import os
import sys

if "/opt/trn_rl_repo" not in sys.path:
    sys.path.insert(0, "/opt/trn_rl_repo")

def _envint(name, default):
    return int(os.environ.get(name, default))

import numpy as np
import ml_dtypes

import concourse.bass as bass
import concourse.mybir as mybir
import concourse.tile as tile
from concourse import bacc
from concourse.masks import make_identity
from concourse.bass_utils import run_bass_kernel_spmd

# Problem shapes (hardcoded per harness contract)
B, N, SD, HD, AD = 512, 256, 1024, 512, 256
NCORES = 8
BL = B // NCORES          # 64 batches per core
G = 8                     # batches per group
NG = BL // G              # 8 groups
TPG = G * N // 128        # 16 row-tiles (128 hint-rows) per group
KH = HD // 128            # 4 contraction chunks over HD
EPS = 1e-8

F32 = mybir.dt.float32
BF16 = mybir.dt.bfloat16
FP8 = mybir.dt.float8e4
S_WH = 64.0     # fp8 scale on Wh.T (values ~0.02 would be subnormal)
S_Q = 32.0      # fp8 scale on q
AF = mybir.ActivationFunctionType
ALU = mybir.AluOpType
AX = mybir.AxisListType


_ACT_TABLE = "natural_log_exp_and_others"


def _patch_act_tables():
    """Force bacc's act-table chooser onto a single table that covers every
    activation this kernel uses (Square/Ln/Exp/Copy), so no per-group
    InstLoadActFuncSet reloads are emitted. Positions are preserved (the
    act_func_set_id is positional), non-chosen sets are just emptied."""
    import concourse.hw_specs as hw_specs

    orig = hw_specs.get_activation_tables

    def patched(module_arch):
        tabs = orig(module_arch)
        return {k: (v if k == _ACT_TABLE else set()) for k, v in tabs.items()}

    bacc.get_activation_tables = patched


def build_nc(stage="full"):
    """stage: 'p1'..'p12' stop after that prologue step; 'prologue';
    'proj'; 'scores'; 'softmax'; 'full'."""
    _patch_act_tables()
    if stage.startswith("p") and stage[1:].isdigit():
        cut = int(stage[1:])
    else:
        cut = 99

    nc = bacc.Bacc("TRN2", target_bir_lowering=False, debug=False,
                   num_devices=NCORES)

    wpack = nc.dram_tensor("wpack", [128, 4610], BF16,
                           kind="ExternalInput")
    xnat = nc.dram_tensor("xnat", [NG, 128, TPG, 512], BF16,
                          kind="ExternalInput")
    xt = nc.dram_tensor("xt", [NG, 128, TPG, KH, 128], FP8,
                        kind="ExternalInput")
    bsb = nc.dram_tensor("bsb", [BL, AD], F32, kind="ExternalInput")
    bhb = nc.dram_tensor("bhb", [BL, AD], F32, kind="ExternalInput")
    out = nc.dram_tensor("out", [BL, HD], F32, kind="ExternalOutput")

    with tile.TileContext(nc) as tc:
        with (
            tc.tile_pool(name="singles", bufs=1) as singles,
            tc.tile_pool(name="xpool", bufs=_envint("KB_XPOOL", 4)) as xpool,
            tc.tile_pool(name="work", bufs=_envint("KB_WORK", 3)) as work,
            tc.tile_pool(name="scratch", bufs=_envint("KB_SCRATCH", 4)) as scratch,
            tc.tile_pool(name="dram", bufs=1, space="DRAM") as dram,
            tc.tile_pool(name="psz", bufs=_envint("KB_PSZ", 4), space="PSUM") as psz_pool,
            tc.tile_pool(name="pss", bufs=_envint("KB_PSS", 2), space="PSUM") as pss_pool,
            tc.tile_pool(name="psw", bufs=_envint("KB_PSW", 2), space="PSUM") as psw_pool,
        ):
            # ---------------- prologue ----------------
            ident = singles.tile([128, 128], F32)
            make_identity(nc, ident)
            ones128 = singles.tile([128, 1], F32)
            nc.vector.memset(ones128[:], 1.0)

            # PE warm-up: fill the otherwise-idle startup DMA window with
            # tiny data-independent matmuls so the HAM clock gate is at
            # full rate when the first real matmuls arrive.
            n_warm = _envint("KB_WARM", 24)
            if n_warm:
                warm_ps = pss_pool.tile([128, 16], F32, tag="pt",
                                        name="warm")
                for i in range(n_warm):
                    nc.tensor.matmul(warm_ps, lhsT=ident,
                                     rhs=ident[:, 0:16],
                                     start=True, stop=True)

            # load small tensors
            wp = singles.tile([128, 4610], BF16)
            nc.sync.dma_start(out=wp[:, 0:2560], in_=wpack[:, 0:2560])
            nc.sync.dma_start(out=wp[:, 2560:], in_=wpack[:, 2560:])
            stateT = wp[:, 0:512].rearrange("p (k b) -> p k b", b=BL)
            wsT = wp[:, 512:2560].rearrange("p (k a) -> p k a", a=AD)
            wh2_sb = wp[:, 2560:3584].rearrange("p (c h) -> p c h", h=HD)
            whT_sb = wp[:, 3584:4608].rearrange("p (k a) -> p k a", a=AD)
            bh2_sb = wp[:, 4608:4610]
            bsb_sb = singles.tile([BL, AD], F32)
            nc.scalar.dma_start(out=bsb_sb[:], in_=bsb[:])
            bhb_sb = singles.tile([BL, AD], F32)
            nc.scalar.dma_start(out=bhb_sb[:], in_=bhb[:])

            def _prologue():
                if cut < 3:
                    return None

                # step 3: sp = state @ Ws.T + bs : [64, 256]
                ps_sp = psz_pool.tile([BL, AD], F32, tag="z", name="ps_sp")
                for k in range(SD // 128):
                    nc.tensor.matmul(ps_sp, lhsT=stateT[:, k, :],
                                     rhs=wsT[:, k, :],
                                     start=(k == 0),
                                     stop=(k == SD // 128 - 1))
                sp_sb = singles.tile([BL, AD], F32)
                nc.vector.tensor_tensor(sp_sb[:], ps_sp[:], bsb_sb[:],
                                        ALU.add)
                if cut < 4:
                    return None

                # step 4: rsn = 1 / max(|sp|, eps); spbh = <sp, bh>
                sq_sp = scratch.tile([BL, AD], F32, tag="sq_sp",
                                     name="sq_sp")
                ssq_sp = singles.tile([BL, 1], F32)
                nc.scalar.activation(out=sq_sp[:], in_=sp_sb[:],
                                     func=AF.Square)
                nc.vector.reduce_sum(out=ssq_sp[:], in_=sq_sp[:], axis=AX.X)
                # rsn = 1/max(|sp|, eps) = exp(-0.5*ln(max(ssq, eps^2)))
                # (keeps ACT on the natural_log_exp table: no table reloads)
                sn = singles.tile([BL, 1], F32)
                nc.vector.tensor_scalar_max(out=sn[:], in0=ssq_sp[:],
                                            scalar1=EPS * EPS)
                nc.scalar.activation(out=sn[:], in_=sn[:], func=AF.Ln)
                rsn = singles.tile([BL, 1], F32)
                nc.scalar.activation(out=rsn[:], in_=sn[:], func=AF.Exp,
                                     scale=-0.5)
                spbh = singles.tile([BL, 1], F32)
                dotscr = scratch.tile([BL, AD], F32, tag="sq_sp",
                                      name="dotscr")
                nc.vector.tensor_tensor(dotscr[:], sp_sb[:], bhb_sb[:],
                                        ALU.mult)
                nc.vector.reduce_sum(out=spbh[:], in_=dotscr[:], axis=AX.X)
                if cut < 5:
                    return None

                # step 5: sp.T : [128, 2, 64]
                spT = singles.tile([128, 2, BL], BF16)
                for c in range(2):
                    pst = pss_pool.tile([128, BL], F32, tag="pt",
                                        name=f"pt_sp{c}")
                    nc.tensor.transpose(
                        pst, sp_sb[:, c * 128:(c + 1) * 128],
                        ident[:BL, :BL])
                    nc.vector.tensor_copy(out=spT[:, c, :], in_=pst)
                if cut < 6:
                    return None

                # step 6: q = sp @ Wh : [64, 512]
                ps_q = psz_pool.tile([BL, HD], F32, tag="z", name="ps_q")
                for c in range(2):
                    nc.tensor.matmul(ps_q, lhsT=spT[:, c, :],
                                     rhs=wh2_sb[:, c, :],
                                     start=(c == 0), stop=(c == 1))
                q_sb = singles.tile([BL, HD], F32)
                nc.vector.tensor_copy(out=q_sb[:], in_=ps_q[:])
                if cut < 7:
                    return None

                # step 7: q.T (bf16) : [128, 4, 64]
                qT = singles.tile([128, KH, BL], FP8)
                for k in range(KH):
                    pst = pss_pool.tile([128, BL], F32, tag="pt",
                                        name=f"pt_q{k}")
                    nc.tensor.transpose(pst, q_sb[:, k * 128:(k + 1) * 128],
                                        ident[:BL, :BL])
                    nc.vector.tensor_scalar_mul(out=qT[:, k, :], in0=pst,
                                                scalar1=S_Q)
                if cut < 8:
                    return None

                # step 8: wb = Wh.T @ bh : [128, 4]
                ps_wb = psw_pool.tile([128, KH], F32, tag="wsum",
                                      name="ps_wb")
                for k in range(KH):
                    for c in range(2):
                        nc.tensor.matmul(
                            ps_wb[:, k:k + 1],
                            lhsT=wh2_sb[:, c, k * 128:(k + 1) * 128],
                            rhs=bh2_sb[:, c:c + 1],
                            start=(c == 0), stop=(c == 1))
                if cut < 9:
                    return None

                # step 9: |bh|^2 -> broadcast [128, 1] via DRAM roundtrip
                ps_c = psw_pool.tile([1, 1], F32, tag="wsum", name="ps_c")
                for c in range(2):
                    nc.tensor.matmul(ps_c, lhsT=bh2_sb[:, c:c + 1],
                                     rhs=bh2_sb[:, c:c + 1],
                                     start=(c == 0), stop=(c == 1))
                c_sb = singles.tile([1, 1], F32)
                nc.vector.tensor_copy(out=c_sb[:], in_=ps_c[:])
                c_dram = dram.tile([1, 1], F32)
                nc.sync.dma_start(out=c_dram[:], in_=c_sb[:])
                c_bcast = singles.tile([128, 1], F32)
                nc.sync.dma_start(out=c_bcast[:],
                                  in_=c_dram.to_broadcast([128, 1]))
                if cut < 10:
                    return None

                # step 10: rsn/spbh broadcast over partitions: [128, 64, 2]
                rb_sb = singles.tile([BL, 2], F32)
                nc.vector.tensor_scalar_mul(out=rb_sb[:, 0:1], in0=rsn[:],
                                            scalar1=1.0 / S_Q)
                nc.vector.tensor_scalar_mul(out=rb_sb[:, 1:2], in0=spbh[:],
                                            scalar1=S_Q)
                rb_dram = dram.tile([BL, 2], F32)
                nc.sync.dma_start(out=rb_dram[:], in_=rb_sb[:])
                rb_bcast = singles.tile([128, BL, 2], F32)
                nc.sync.dma_start(
                    out=rb_bcast[:],
                    in_=rb_dram[None].to_broadcast([128, BL, 2]))
                if cut < 11:
                    return None

                # step 11: augmented moving operand [Wh.T | q_b | wb] (bf16)
                rhs_aug = []
                for p in range(2):
                    buf = singles.tile([128, KH, AD + 2], FP8,
                                       tag=f"rhsaug{p}", name=f"rhsaug{p}")
                    rhs_aug.append(buf)
                for p in range(2):
                    nc.vector.tensor_scalar_mul(
                        out=rhs_aug[p][:, :, 0:AD], in0=whT_sb[:],
                        scalar1=S_WH)
                for p in range(2):
                    nc.vector.tensor_scalar_mul(
                        out=rhs_aug[p][:, :, AD + 1:AD + 2],
                        in0=ps_wb[:, :, None], scalar1=S_WH * S_WH)
                if cut < 12:
                    return None

                # step 12: block-diagonal attn holders (bf16)
                attn_bd = []
                for p in range(2):
                    t = singles.tile([128, TPG, G], BF16,
                                     tag=f"attnbd{p}", name=f"attnbd{p}")
                    nc.vector.memset(t[:], 0.0)
                    attn_bd.append(t)

                return dict(qT=qT, rb_bcast=rb_bcast, c_bcast=c_bcast,
                            rhs_aug=rhs_aug, attn_bd=attn_bd)

            pro = _prologue()

            # ---------------- main loop ----------------
            if pro is not None and stage not in ("prologue",):
                qT = pro["qT"]
                rb_bcast = pro["rb_bcast"]
                c_bcast = pro["c_bcast"]
                rhs_aug = pro["rhs_aug"]
                attn_bd = pro["attn_bd"]

                pending = []
                HB = G // 2           # 4 batches per half
                HT = TPG // 2         # 8 tiles per half
                state_h = {}          # group -> dict with group tiles

                def emit_tail(gh):
                    # PE tail of half gh: 8 weighted-sum matmuls; on the
                    # second half also the exp-sum, normalizer and output.
                    g, h = divmod(gh, 2)
                    st = state_h[g]
                    xn, abd, psw, exp_sc = (st["xn"], st["abd"], st["psw"],
                                            st["exp_sc"])
                    for tt in range(HT):
                        t = h * HT + tt
                        nc.tensor.matmul(psw, lhsT=abd[:, t, :],
                                         rhs=xn[:, t, :],
                                         start=(t == 0), stop=(t == TPG - 1),
                                         skip_group_check=True)
                    if h == 0:
                        return
                    ps_se = pss_pool.tile([1, 2 * G], F32, tag="pt",
                                          name=f"ps_se{g}")
                    nc.tensor.matmul(ps_se, lhsT=ones128[:, 0:1],
                                     rhs=exp_sc.rearrange("p b h -> p (b h)"),
                                     start=True, stop=True)
                    se1 = work.tile([1, G], F32, tag="se1", name=f"se1{g}")
                    nc.vector.reduce_sum(
                        out=se1[:], in_=ps_se.rearrange("p (b h) -> p b h",
                                                        h=2), axis=AX.X)
                    ps_set = pss_pool.tile([G, 1], F32, tag="pt",
                                           name=f"ps_set{g}")
                    nc.tensor.matmul(ps_set, lhsT=se1[:], rhs=ident[:1, :1],
                                     start=True, stop=True)
                    rse = work.tile([G, 1], F32, tag="rse", name=f"rse{g}")
                    nc.vector.reciprocal(out=rse[:], in_=ps_set[:])
                    outg = work.tile([G, 512], F32, tag="outg",
                                     name=f"outg{g}")
                    nc.vector.tensor_scalar_mul(out=outg[:], in0=psw[:],
                                                scalar1=rse[:])
                    nc.scalar.dma_start(out=out[g * G:(g + 1) * G, :],
                                        in_=outg[:])
                    del state_h[g]

                for gh in range(2 * NG):
                    g, h = divmod(gh, 2)
                    if h == 0:
                        xtt = xpool.tile([128, TPG, KH, 128], FP8,
                                         tag="xt", name=f"xt{g}")
                        xn = xpool.tile([128, TPG, 512], BF16, tag="xnat",
                                        name=f"xn{g}")
                        ssq_g = work.tile([128, TPG], F32, tag="ssq",
                                          name=f"ssq{g}")
                        zqw_g = work.tile([128, TPG, 2], F32, tag="zqw",
                                          name=f"zqw{g}")
                        exp_sc = work.tile([128, G, 2], F32, tag="expsc",
                                           name=f"expsc{g}")
                        abd = attn_bd[g % 2]
                        psw = psw_pool.tile([G, 512], F32, tag="wsum",
                                            name=f"psw{g}")
                        state_h[g] = dict(xtt=xtt, xn=xn, ssq=ssq_g,
                                          zqw=zqw_g, exp_sc=exp_sc, abd=abd,
                                          psw=psw)
                    st = state_h[g]
                    xtt, xn, ssq_g, zqw_g, exp_sc, abd = (
                        st["xtt"], st["xn"], st["ssq"], st["zqw"],
                        st["exp_sc"], st["abd"])

                    # loads for this half
                    _qs = _envint("KB_QSPLIT", 2)
                    _step = HT // _qs
                    for _q in range(_qs):
                        _a = h * HT + _q * _step
                        nc.sync.dma_start(out=xtt[:, _a:_a + _step],
                                          in_=xt[g][:, _a:_a + _step])
                    for _q in range(_qs):
                        _a = h * HT + _q * _step
                        nc.sync.dma_start(out=xn[:, _a:_a + _step],
                                          in_=xnat[g][:, _a:_a + _step])

                    # projection for 4 batches
                    for bl in range(h * HB, (h + 1) * HB):
                        b = g * G + bl
                        buf = rhs_aug[b % 2]
                        nc.gpsimd.tensor_copy(out=buf[:, :, AD:AD + 1],
                                              in_=qT[:, :, b:b + 1])
                        for t2 in range(2):
                            t = bl * 2 + t2
                            psz = psz_pool.tile([128, AD + 2], F32, tag="z",
                                                name=f"z{g}_{t}")
                            for k in range(KH):
                                nc.tensor.matmul(psz, lhsT=xtt[:, t, k, :],
                                                 rhs=buf[:, k, :],
                                                 start=(k == 0),
                                                 stop=(k == KH - 1))
                            sq = scratch.tile([128, AD], BF16, tag="sq",
                                              name=f"sq{g}_{t}")
                            _sqpat = _envint('KB_SQPAT', 3)
                            if _sqpat == 0:
                                _dve_sq = (t % 3 == 2)
                            elif _sqpat == 1:
                                _dve_sq = (t % 8 >= 6)
                            elif _sqpat == 2:
                                _dve_sq = (t % 8 >= 5)
                            elif _sqpat == 3:
                                _dve_sq = (t % 4 == 3)
                            else:
                                _dve_sq = (t % 8 in (3, 6, 7))
                            if _dve_sq:
                                nc.vector.tensor_scalar(
                                    out=sq[:], in0=psz[:, 0:AD],
                                    scalar1=1.0 / S_WH, scalar2=None,
                                    op0=ALU.mult)
                                nc.vector.tensor_tensor(sq[:], sq[:], sq[:],
                                                        ALU.mult)
                                nc.vector.reduce_sum(
                                    out=ssq_g[:, t:t + 1], in_=sq[:],
                                    axis=AX.X)
                            else:
                                nc.scalar.activation(
                                    out=sq[:], in_=psz[:, 0:AD],
                                    func=AF.Square, scale=1.0 / S_WH,
                                    accum_out=ssq_g[:, t:t + 1])
                            if _envint('KB_ZQACT', 0) and t % 2:
                                nc.scalar.copy(out=zqw_g[:, t, :],
                                               in_=psz[:, AD:AD + 2])
                            else:
                                nc.vector.tensor_copy(
                                    out=zqw_g[:, t, :],
                                    in_=psz[:, AD:AD + 2])

                    if stage == "proj":
                        continue

                    # ---- epilogue for this half: norms and scores ----
                    hs = slice(h * HT, (h + 1) * HT)
                    hn2 = work.tile([128, HT], F32, tag="hn2",
                                    name=f"hn2_{gh}")
                    nc.vector.tensor_scalar(
                        out=hn2[:], in0=zqw_g[:, hs, 1],
                        scalar1=2.0 / (S_WH * S_WH), scalar2=c_bcast[:],
                        op0=ALU.mult, op1=ALU.add)
                    nc.vector.tensor_tensor(hn2[:], hn2[:], ssq_g[:, hs],
                                            ALU.add)
                    # rhn = 1/max(|hp|, eps) = exp(-0.5*ln(max(hn2, eps^2)))
                    nc.vector.tensor_scalar_max(out=hn2[:], in0=hn2[:],
                                                scalar1=EPS * EPS)
                    nc.scalar.activation(out=hn2[:], in_=hn2[:], func=AF.Ln)
                    rhn = work.tile([128, HT], F32, tag="rhn",
                                    name=f"rhn{gh}")
                    nc.scalar.activation(out=rhn[:], in_=hn2[:], func=AF.Exp,
                                         scale=-0.5)

                    scores = work.tile([128, HB, 2], F32, tag="scores",
                                       name=f"scores{gh}")
                    zq_v = zqw_g[:, hs, 0].rearrange("p (b h) -> p b h", h=2)
                    rhn_v = rhn.rearrange("p (b h) -> p b h", h=2)
                    b0 = g * G + h * HB
                    spbh_rep = rb_bcast[:, b0:b0 + HB, 1:2].to_broadcast(
                        [128, HB, 2])
                    rsn_rep = rb_bcast[:, b0:b0 + HB, 0:1].to_broadcast(
                        [128, HB, 2])
                    nc.vector.tensor_tensor(scores[:], zq_v, spbh_rep,
                                            ALU.add)
                    nc.vector.tensor_tensor(scores[:], scores[:], rsn_rep,
                                            ALU.mult)
                    nc.vector.tensor_tensor(scores[:], scores[:], rhn_v,
                                            ALU.mult)

                    if stage == "scores":
                        continue

                    # exp(scores): cosine sims are in [-1,1], no
                    # max-subtraction needed; normalizer rides the wsum
                    nc.scalar.activation(
                        out=exp_sc[:, h * HB:(h + 1) * HB, :], in_=scores[:],
                        func=AF.Exp)

                    # scatter exp(s) into the block-diagonal stationary
                    abd_flat = abd.rearrange("p t b -> p (t b)")
                    stride = 2 * G + 1
                    for t2 in range(2):
                        s0 = h * HB * stride + t2 * G
                        nc.vector.tensor_copy(
                            out=abd_flat[:, s0:s0 + (HB - 1) * stride
                                         + 1:stride],
                            in_=exp_sc[:, h * HB:(h + 1) * HB, t2])

                    pending.append(gh)
                    _pend = _envint('KB_PEND', 3)
                    if gh >= 2 * NG - _envint('KB_PENDTAIL', 0):
                        _pend = 1
                    while len(pending) > _pend:
                        emit_tail(pending.pop(0))

                while pending:
                    emit_tail(pending.pop(0))

    nc.compile()
    return nc


_NC = None


def _get_nc():
    global _NC
    if _NC is None:
        _NC = build_nc()
    return _NC


def _prep_core_inputs(state_emb, hints_emb, Ws, bs, Wh, bh, core):
    bf16 = ml_dtypes.bfloat16
    s = slice(core * BL, (core + 1) * BL)
    hf = np.ascontiguousarray(hints_emb[s]).reshape(BL * N, HD)
    hfb = hf.astype(bf16)
    hf8 = hf.astype(ml_dtypes.float8_e4m3)
    # natural: (g, p, t, f) with row = g*2048 + t*128 + p
    xnat = np.ascontiguousarray(
        hfb.reshape(NG, TPG, 128, 512).transpose(0, 2, 1, 3))
    # transposed: (g, p, t, k, r) with row = g*2048 + t*128 + r, h = k*128+p
    xtd = np.ascontiguousarray(
        hf8.reshape(NG, TPG, 128, KH, 128).transpose(0, 4, 1, 3, 2))
    bf = ml_dtypes.bfloat16
    # Ws.T arranged [s_in_chunk, s_chunk, a]
    wst = Ws.T.reshape(SD // 128, 128, AD).transpose(1, 0, 2)
    wh2 = Wh.reshape(2, 128, HD).transpose(1, 0, 2)
    # Wh.T arranged [h_in_chunk, h_chunk, a]
    wht = Wh.T.reshape(KH, 128, AD).transpose(1, 0, 2)
    bh2 = bh.reshape(2, 128).T
    bsb = np.ascontiguousarray(
        np.broadcast_to(bs, (BL, AD))).astype(np.float32)
    bhb = np.ascontiguousarray(
        np.broadcast_to(bh, (BL, AD))).astype(np.float32)
    st = np.asarray(state_emb[s])
    # state.T arranged [s_in_chunk, s_chunk, b]
    statet = st.T.reshape(SD // 128, 128, BL).transpose(1, 0, 2)
    wpack = np.concatenate([
        statet.reshape(128, -1), wst.reshape(128, -1),
        wh2.reshape(128, -1), wht.reshape(128, -1),
        bh2.reshape(128, -1),
    ], axis=1).astype(bf)
    wpack = np.ascontiguousarray(wpack)
    return {
        "wpack": wpack,
        "xnat": xnat,
        "xt": xtd,
        "bsb": bsb,
        "bhb": bhb,
    }


def kernel(state_emb, hints_emb, Ws, bs, Wh, bh):
    state_emb = np.asarray(state_emb, dtype=np.float32)
    hints_emb = np.asarray(hints_emb, dtype=np.float32)
    Ws = np.asarray(Ws, dtype=np.float32)
    bs = np.asarray(bs, dtype=np.float32)
    Wh = np.asarray(Wh, dtype=np.float32)
    bh = np.asarray(bh, dtype=np.float32)

    nc = _get_nc()
    in_maps = [
        _prep_core_inputs(state_emb, hints_emb, Ws, bs, Wh, bh, c)
        for c in range(NCORES)
    ]
    res = run_bass_kernel_spmd(nc, in_maps, core_ids=list(range(NCORES)))
    return np.concatenate([res.results[c]["out"] for c in range(NCORES)],
                          axis=0)



# revision 2
# speedup vs baseline: 1.0550x; 1.0550x over previous
"""Trainium2 Bass kernel for cosine-similarity hint attention.

Computation (per batch b):
  sp = state_emb @ Ws.T + bs                  (B, A)
  hp = hints_emb @ Wh.T + bh                  (B, N, A)
  scores = <sp, hp> / (max(|sp|,eps) * max(|hp|,eps))
  attn = softmax(scores, axis=N)
  out = attn @ hints_emb                      (B, HD)

Distribution: data-parallel over batch, B=512 -> 64 per core on 8 cores.
Weights replicated. No collectives.

Device-side algorithm (per core, pipelined over 16 half-groups of 4
batches = 8 row-tiles of 128 hint-rows each):
  - hints arrive in two host-prepared layouts: natural [r, h] tiles in
    bf16 (for the final weighted sum, contraction over rows) and
    transposed [h, r] tiles in fp8-e4m3 (for the hint projection,
    contraction over h). Host-side work is layout/precision only.
  - hint projection z' = X @ (S*Wh.T) runs on TensorE with an augmented
    fp8 moving operand [S*Wh.T | S_q*q_b | S^2*wb] where q_b = sp_b @ Wh
    and wb = Wh.T @ bh; scales dodge fp8 subnormals and are divided out
    for free downstream. One pass yields z (256 cols), zq = <sp_b, z_r>
    and zw = <z_r, bh>.
  - |hp_r|^2 = sum((z'/S)^2) + (2/S^2)*zw' + |bh|^2; the squares run on
    ScalarE Square(scale=1/S)+accumulate (3 of every 4 tiles) and
    VectorE mult+reduce (the trailing tile of each 4), keeping both
    engines under the TensorE/DMA pace.
  - 1/max(norm, eps) = exp(-0.5*ln(max(norm^2, eps^2))) so every
    activation (Square/Ln/Exp) lives in one ACT table (no reloads).
  - softmax needs no max-subtraction (cosine scores are in [-1,1]);
    exp(scores) is scattered straight into a block-diagonal bf16
    stationary (stride-17 writes) so all 8 batches of a group
    accumulate in one [8, 512] PSUM tile over 16 weighted-sum matmuls;
    the normalizer (ones-vector matmul + pair-reduce) divides at the
    output copy.
  - emission is software-pipelined: each half-group's PE-heavy tail
    (weighted sum + normalizer) is deferred 3 halves so TensorE never
    head-of-line blocks on the DVE/ACT epilogue chain.
"""

import os
import sys

if "/opt/trn_rl_repo" not in sys.path:
    sys.path.insert(0, "/opt/trn_rl_repo")

def _envint(name, default):
    return int(os.environ.get(name, default))

import numpy as np
import ml_dtypes

import concourse.bass as bass
import concourse.mybir as mybir
import concourse.tile as tile
from concourse import bacc
from concourse.masks import make_identity
from concourse.bass_utils import run_bass_kernel_spmd

# Problem shapes (hardcoded per harness contract)
B, N, SD, HD, AD = 512, 256, 1024, 512, 256
NCORES = 8
BL = B // NCORES          # 64 batches per core
G = 8                     # batches per group
NG = BL // G              # 8 groups
TPG = G * N // 128        # 16 row-tiles (128 hint-rows) per group
KH = HD // 128            # 4 contraction chunks over HD
EPS = 1e-8

F32 = mybir.dt.float32
BF16 = mybir.dt.bfloat16
FP8 = mybir.dt.float8e4
S_WH = 64.0     # fp8 scale on Wh.T (values ~0.02 would be subnormal)
S_Q = 32.0      # fp8 scale on q
AF = mybir.ActivationFunctionType
ALU = mybir.AluOpType
AX = mybir.AxisListType


_ACT_TABLE = "natural_log_exp_and_others"


def _patch_act_tables():
    """Force bacc's act-table chooser onto a single table that covers every
    activation this kernel uses (Square/Ln/Exp/Copy), so no per-group
    InstLoadActFuncSet reloads are emitted. Positions are preserved (the
    act_func_set_id is positional), non-chosen sets are just emptied."""
    import concourse.hw_specs as hw_specs

    orig = hw_specs.get_activation_tables

    def patched(module_arch):
        tabs = orig(module_arch)
        return {k: (v if k == _ACT_TABLE else set()) for k, v in tabs.items()}

    bacc.get_activation_tables = patched


def build_nc(stage="full"):
    """stage: 'p1'..'p12' stop after that prologue step; 'prologue';
    'proj'; 'scores'; 'softmax'; 'full'."""
    _patch_act_tables()
    if stage.startswith("p") and stage[1:].isdigit():
        cut = int(stage[1:])
    else:
        cut = 99

    nc = bacc.Bacc("TRN2", target_bir_lowering=False, debug=False,
                   num_devices=NCORES)

    wpack = nc.dram_tensor("wpack", [128, 4610], BF16,
                           kind="ExternalInput")
    xnat = nc.dram_tensor("xnat", [NG, 128, TPG, 512], BF16,
                          kind="ExternalInput")
    xt = nc.dram_tensor("xt", [NG, 128, TPG, KH, 128], FP8,
                        kind="ExternalInput")
    bsb = nc.dram_tensor("bsb", [BL, AD], F32, kind="ExternalInput")
    bhb = nc.dram_tensor("bhb", [BL, AD], F32, kind="ExternalInput")
    out = nc.dram_tensor("out", [BL, HD], F32, kind="ExternalOutput")

    with tile.TileContext(nc) as tc:
        with (
            tc.tile_pool(name="singles", bufs=1) as singles,
            tc.tile_pool(name="xpool", bufs=_envint("KB_XPOOL", 4)) as xpool,
            tc.tile_pool(name="work", bufs=_envint("KB_WORK", 3)) as work,
            tc.tile_pool(name="scratch", bufs=_envint("KB_SCRATCH", 4)) as scratch,
            tc.tile_pool(name="dram", bufs=1, space="DRAM") as dram,
            tc.tile_pool(name="psz", bufs=_envint("KB_PSZ", 4), space="PSUM") as psz_pool,
            tc.tile_pool(name="pss", bufs=_envint("KB_PSS", 2), space="PSUM") as pss_pool,
            tc.tile_pool(name="psw", bufs=_envint("KB_PSW", 2), space="PSUM") as psw_pool,
        ):
            # ---------------- prologue ----------------
            ident = singles.tile([128, 128], F32)
            make_identity(nc, ident)
            ones128 = singles.tile([128, 1], F32)
            nc.vector.memset(ones128[:], 1.0)

            # PE warm-up: fill the otherwise-idle startup DMA window with
            # tiny data-independent matmuls so the HAM clock gate is at
            # full rate when the first real matmuls arrive.
            n_warm = _envint("KB_WARM", 24)
            if n_warm:
                warm_ps = pss_pool.tile([128, 16], F32, tag="pt",
                                        name="warm")
                for i in range(n_warm):
                    nc.tensor.matmul(warm_ps, lhsT=ident,
                                     rhs=ident[:, 0:16],
                                     start=True, stop=True)

            # load small tensors
            wp = singles.tile([128, 4610], BF16)
            nc.sync.dma_start(out=wp[:, 0:2560], in_=wpack[:, 0:2560])
            nc.sync.dma_start(out=wp[:, 2560:], in_=wpack[:, 2560:])
            stateT = wp[:, 0:512].rearrange("p (k b) -> p k b", b=BL)
            wsT = wp[:, 512:2560].rearrange("p (k a) -> p k a", a=AD)
            wh2_sb = wp[:, 2560:3584].rearrange("p (c h) -> p c h", h=HD)
            whT_sb = wp[:, 3584:4608].rearrange("p (k a) -> p k a", a=AD)
            bh2_sb = wp[:, 4608:4610]
            bsb_sb = singles.tile([BL, AD], F32)
            nc.scalar.dma_start(out=bsb_sb[:], in_=bsb[:])
            bhb_sb = singles.tile([BL, AD], F32)
            nc.scalar.dma_start(out=bhb_sb[:], in_=bhb[:])

            def _prologue():
                if cut < 3:
                    return None

                # step 3: sp = state @ Ws.T + bs : [64, 256]
                ps_sp = psz_pool.tile([BL, AD], F32, tag="z", name="ps_sp")
                for k in range(SD // 128):
                    nc.tensor.matmul(ps_sp, lhsT=stateT[:, k, :],
                                     rhs=wsT[:, k, :],
                                     start=(k == 0),
                                     stop=(k == SD // 128 - 1))
                sp_sb = singles.tile([BL, AD], F32)
                nc.vector.tensor_tensor(sp_sb[:], ps_sp[:], bsb_sb[:],
                                        ALU.add)
                if cut < 4:
                    return None

                # step 4: rsn = 1 / max(|sp|, eps); spbh = <sp, bh>
                sq_sp = scratch.tile([BL, AD], F32, tag="sq_sp",
                                     name="sq_sp")
                ssq_sp = singles.tile([BL, 1], F32)
                nc.scalar.activation(out=sq_sp[:], in_=sp_sb[:],
                                     func=AF.Square)
                nc.vector.reduce_sum(out=ssq_sp[:], in_=sq_sp[:], axis=AX.X)
                # rsn = 1/max(|sp|, eps) = exp(-0.5*ln(max(ssq, eps^2)))
                # (keeps ACT on the natural_log_exp table: no table reloads)
                sn = singles.tile([BL, 1], F32)
                nc.vector.tensor_scalar_max(out=sn[:], in0=ssq_sp[:],
                                            scalar1=EPS * EPS)
                nc.scalar.activation(out=sn[:], in_=sn[:], func=AF.Ln)
                rsn = singles.tile([BL, 1], F32)
                nc.scalar.activation(out=rsn[:], in_=sn[:], func=AF.Exp,
                                     scale=-0.5)
                spbh = singles.tile([BL, 1], F32)
                dotscr = scratch.tile([BL, AD], F32, tag="sq_sp",
                                      name="dotscr")
                nc.vector.tensor_tensor(dotscr[:], sp_sb[:], bhb_sb[:],
                                        ALU.mult)
                nc.vector.reduce_sum(out=spbh[:], in_=dotscr[:], axis=AX.X)
                if cut < 5:
                    return None

                # step 5: sp.T : [128, 2, 64]
                spT = singles.tile([128, 2, BL], BF16)
                for c in range(2):
                    pst = pss_pool.tile([128, BL], F32, tag="pt",
                                        name=f"pt_sp{c}")
                    nc.tensor.transpose(
                        pst, sp_sb[:, c * 128:(c + 1) * 128],
                        ident[:BL, :BL])
                    nc.vector.tensor_copy(out=spT[:, c, :], in_=pst)
                if cut < 6:
                    return None

                # step 6: q = sp @ Wh : [64, 512]
                ps_q = psz_pool.tile([BL, HD], F32, tag="z", name="ps_q")
                for c in range(2):
                    nc.tensor.matmul(ps_q, lhsT=spT[:, c, :],
                                     rhs=wh2_sb[:, c, :],
                                     start=(c == 0), stop=(c == 1))
                q_sb = singles.tile([BL, HD], F32)
                nc.vector.tensor_copy(out=q_sb[:], in_=ps_q[:])
                if cut < 7:
                    return None

                # step 7: q.T (bf16) : [128, 4, 64]
                qT = singles.tile([128, KH, BL], FP8)
                for k in range(KH):
                    pst = pss_pool.tile([128, BL], F32, tag="pt",
                                        name=f"pt_q{k}")
                    nc.tensor.transpose(pst, q_sb[:, k * 128:(k + 1) * 128],
                                        ident[:BL, :BL])
                    nc.vector.tensor_scalar_mul(out=qT[:, k, :], in0=pst,
                                                scalar1=S_Q)
                if cut < 8:
                    return None

                # step 8: wb = Wh.T @ bh : [128, 4]
                ps_wb = psw_pool.tile([128, KH], F32, tag="wsum",
                                      name="ps_wb")
                for k in range(KH):
                    for c in range(2):
                        nc.tensor.matmul(
                            ps_wb[:, k:k + 1],
                            lhsT=wh2_sb[:, c, k * 128:(k + 1) * 128],
                            rhs=bh2_sb[:, c:c + 1],
                            start=(c == 0), stop=(c == 1))
                if cut < 9:
                    return None

                # step 9: |bh|^2 -> broadcast [128, 1] via DRAM roundtrip
                ps_c = psw_pool.tile([1, 1], F32, tag="wsum", name="ps_c")
                for c in range(2):
                    nc.tensor.matmul(ps_c, lhsT=bh2_sb[:, c:c + 1],
                                     rhs=bh2_sb[:, c:c + 1],
                                     start=(c == 0), stop=(c == 1))
                c_sb = singles.tile([1, 1], F32)
                nc.vector.tensor_copy(out=c_sb[:], in_=ps_c[:])
                c_dram = dram.tile([1, 1], F32)
                nc.sync.dma_start(out=c_dram[:], in_=c_sb[:])
                c_bcast = singles.tile([128, 1], F32)
                nc.sync.dma_start(out=c_bcast[:],
                                  in_=c_dram.to_broadcast([128, 1]))
                if cut < 10:
                    return None

                # step 10: rsn/spbh broadcast over partitions: [128, 64, 2]
                rb_sb = singles.tile([BL, 2], F32)
                nc.vector.tensor_scalar_mul(out=rb_sb[:, 0:1], in0=rsn[:],
                                            scalar1=1.0 / S_Q)
                nc.vector.tensor_scalar_mul(out=rb_sb[:, 1:2], in0=spbh[:],
                                            scalar1=S_Q)
                rb_dram = dram.tile([BL, 2], F32)
                nc.sync.dma_start(out=rb_dram[:], in_=rb_sb[:])
                rb_bcast = singles.tile([128, BL, 2], F32)
                nc.sync.dma_start(
                    out=rb_bcast[:],
                    in_=rb_dram[None].to_broadcast([128, BL, 2]))
                if cut < 11:
                    return None

                # step 11: augmented moving operand [Wh.T | q_b | wb] (bf16)
                rhs_aug = []
                for p in range(2):
                    buf = singles.tile([128, KH, AD + 2], FP8,
                                       tag=f"rhsaug{p}", name=f"rhsaug{p}")
                    rhs_aug.append(buf)
                for p in range(2):
                    nc.vector.tensor_scalar_mul(
                        out=rhs_aug[p][:, :, 0:AD], in0=whT_sb[:],
                        scalar1=S_WH)
                for p in range(2):
                    nc.vector.tensor_scalar_mul(
                        out=rhs_aug[p][:, :, AD + 1:AD + 2],
                        in0=ps_wb[:, :, None], scalar1=S_WH * S_WH)
                if cut < 12:
                    return None

                # step 12: block-diagonal attn holders (bf16)
                attn_bd = []
                for p in range(2):
                    t = singles.tile([128, TPG, G], BF16,
                                     tag=f"attnbd{p}", name=f"attnbd{p}")
                    nc.vector.memset(t[:], 0.0)
                    attn_bd.append(t)

                return dict(qT=qT, rb_bcast=rb_bcast, c_bcast=c_bcast,
                            rhs_aug=rhs_aug, attn_bd=attn_bd)

            pro = _prologue()

            # ---------------- main loop ----------------
            if pro is not None and stage not in ("prologue",):
                qT = pro["qT"]
                rb_bcast = pro["rb_bcast"]
                c_bcast = pro["c_bcast"]
                rhs_aug = pro["rhs_aug"]
                attn_bd = pro["attn_bd"]

                pending = []
                HB = G // 2           # 4 batches per half
                HT = TPG // 2         # 8 tiles per half
                state_h = {}          # group -> dict with group tiles

                def emit_tail(gh):
                    # PE tail of half gh: 8 weighted-sum matmuls; on the
                    # second half also the exp-sum, normalizer and output.
                    g, h = divmod(gh, 2)
                    st = state_h[g]
                    xn, abd, psw, exp_sc = (st["xn"], st["abd"], st["psw"],
                                            st["exp_sc"])
                    for tt in range(HT):
                        t = h * HT + tt
                        nc.tensor.matmul(psw, lhsT=abd[:, t, :],
                                         rhs=xn[:, t, :],
                                         start=(t == 0), stop=(t == TPG - 1),
                                         skip_group_check=True)
                    if h == 0:
                        return
                    ps_se = pss_pool.tile([1, 2 * G], F32, tag="pt",
                                          name=f"ps_se{g}")
                    nc.tensor.matmul(ps_se, lhsT=ones128[:, 0:1],
                                     rhs=exp_sc.rearrange("p b h -> p (b h)"),
                                     start=True, stop=True)
                    se1 = work.tile([1, G], F32, tag="se1", name=f"se1{g}")
                    nc.vector.reduce_sum(
                        out=se1[:], in_=ps_se.rearrange("p (b h) -> p b h",
                                                        h=2), axis=AX.X)
                    ps_set = pss_pool.tile([G, 1], F32, tag="pt",
                                           name=f"ps_set{g}")
                    nc.tensor.matmul(ps_set, lhsT=se1[:], rhs=ident[:1, :1],
                                     start=True, stop=True)
                    rse = work.tile([G, 1], F32, tag="rse", name=f"rse{g}")
                    nc.vector.reciprocal(out=rse[:], in_=ps_set[:])
                    outg = work.tile([G, 512], F32, tag="outg",
                                     name=f"outg{g}")
                    nc.vector.tensor_scalar_mul(out=outg[:], in0=psw[:],
                                                scalar1=rse[:])
                    nc.scalar.dma_start(out=out[g * G:(g + 1) * G, :],
                                        in_=outg[:])
                    del state_h[g]

                for gh in range(2 * NG):
                    g, h = divmod(gh, 2)
                    if h == 0:
                        xtt = xpool.tile([128, TPG, KH, 128], FP8,
                                         tag="xt", name=f"xt{g}")
                        xn = xpool.tile([128, TPG, 512], BF16, tag="xnat",
                                        name=f"xn{g}")
                        ssq_g = work.tile([128, TPG], F32, tag="ssq",
                                          name=f"ssq{g}")
                        zqw_g = work.tile([128, TPG, 2], F32, tag="zqw",
                                          name=f"zqw{g}")
                        exp_sc = work.tile([128, G, 2], F32, tag="expsc",
                                           name=f"expsc{g}")
                        abd = attn_bd[g % 2]
                        psw = psw_pool.tile([G, 512], F32, tag="wsum",
                                            name=f"psw{g}")
                        state_h[g] = dict(xtt=xtt, xn=xn, ssq=ssq_g,
                                          zqw=zqw_g, exp_sc=exp_sc, abd=abd,
                                          psw=psw)
                    st = state_h[g]
                    xtt, xn, ssq_g, zqw_g, exp_sc, abd = (
                        st["xtt"], st["xn"], st["ssq"], st["zqw"],
                        st["exp_sc"], st["abd"])

                    # loads for this half
                    _qs = _envint("KB_QSPLIT", 2)
                    _step = HT // _qs
                    for _q in range(_qs):
                        _a = h * HT + _q * _step
                        nc.sync.dma_start(out=xtt[:, _a:_a + _step],
                                          in_=xt[g][:, _a:_a + _step])
                    for _q in range(_qs):
                        _a = h * HT + _q * _step
                        nc.sync.dma_start(out=xn[:, _a:_a + _step],
                                          in_=xnat[g][:, _a:_a + _step])

                    # projection for 4 batches (fp8 DoubleRow: 2 k-tiles per
                    # matmul, 0.5 cycles/row)
                    for bl in range(h * HB, (h + 1) * HB):
                        b = g * G + bl
                        buf = rhs_aug[b % 2]
                        buf_dr = buf.rearrange("p (kk i) a -> p kk i a", kk=2)
                        nc.gpsimd.tensor_copy(out=buf[:, :, AD:AD + 1],
                                              in_=qT[:, :, b:b + 1])
                        for t2 in range(2):
                            t = bl * 2 + t2
                            xt_dr = xtt[:, t].rearrange(
                                "p (kk i) r -> p kk i r", kk=2)
                            psz = psz_pool.tile([128, AD + 2], F32, tag="z",
                                                name=f"z{g}_{t}")
                            for kk in range(2):
                                nc.tensor.matmul(
                                    psz, lhsT=xt_dr[:, kk], rhs=buf_dr[:, kk],
                                    start=(kk == 0), stop=(kk == 1),
                                    perf_mode=mybir.MatmulPerfMode.DoubleRow)
                            sq = scratch.tile([128, AD], BF16, tag="sq",
                                              name=f"sq{g}_{t}")
                            _sqpat = _envint('KB_SQPAT', 3)
                            if _sqpat == 0:
                                _dve_sq = (t % 3 == 2)
                            elif _sqpat == 1:
                                _dve_sq = (t % 8 >= 6)
                            elif _sqpat == 2:
                                _dve_sq = (t % 8 >= 5)
                            elif _sqpat == 3:
                                _dve_sq = (t % 4 == 3)
                            else:
                                _dve_sq = (t % 8 in (3, 6, 7))
                            if _dve_sq:
                                nc.vector.tensor_scalar(
                                    out=sq[:], in0=psz[:, 0:AD],
                                    scalar1=1.0 / S_WH, scalar2=None,
                                    op0=ALU.mult)
                                nc.vector.tensor_tensor(sq[:], sq[:], sq[:],
                                                        ALU.mult)
                                nc.vector.reduce_sum(
                                    out=ssq_g[:, t:t + 1], in_=sq[:],
                                    axis=AX.X)
                            else:
                                nc.scalar.activation(
                                    out=sq[:], in_=psz[:, 0:AD],
                                    func=AF.Square, scale=1.0 / S_WH,
                                    accum_out=ssq_g[:, t:t + 1])
                            if _envint('KB_ZQACT', 0) and t % 2:
                                nc.scalar.copy(out=zqw_g[:, t, :],
                                               in_=psz[:, AD:AD + 2])
                            else:
                                nc.vector.tensor_copy(
                                    out=zqw_g[:, t, :],
                                    in_=psz[:, AD:AD + 2])

                    if stage == "proj":
                        continue

                    # ---- epilogue for this half: norms and scores ----
                    hs = slice(h * HT, (h + 1) * HT)
                    hn2 = work.tile([128, HT], F32, tag="hn2",
                                    name=f"hn2_{gh}")
                    nc.vector.tensor_scalar(
                        out=hn2[:], in0=zqw_g[:, hs, 1],
                        scalar1=2.0 / (S_WH * S_WH), scalar2=c_bcast[:],
                        op0=ALU.mult, op1=ALU.add)
                    nc.vector.tensor_tensor(hn2[:], hn2[:], ssq_g[:, hs],
                                            ALU.add)
                    # rhn = 1/max(|hp|, eps) = exp(-0.5*ln(max(hn2, eps^2)))
                    nc.vector.tensor_scalar_max(out=hn2[:], in0=hn2[:],
                                                scalar1=EPS * EPS)
                    nc.scalar.activation(out=hn2[:], in_=hn2[:], func=AF.Ln)
                    rhn = work.tile([128, HT], F32, tag="rhn",
                                    name=f"rhn{gh}")
                    nc.scalar.activation(out=rhn[:], in_=hn2[:], func=AF.Exp,
                                         scale=-0.5)

                    scores = work.tile([128, HB, 2], F32, tag="scores",
                                       name=f"scores{gh}")
                    zq_v = zqw_g[:, hs, 0].rearrange("p (b h) -> p b h", h=2)
                    rhn_v = rhn.rearrange("p (b h) -> p b h", h=2)
                    b0 = g * G + h * HB
                    spbh_rep = rb_bcast[:, b0:b0 + HB, 1:2].to_broadcast(
                        [128, HB, 2])
                    rsn_rep = rb_bcast[:, b0:b0 + HB, 0:1].to_broadcast(
                        [128, HB, 2])
                    nc.vector.tensor_tensor(scores[:], zq_v, spbh_rep,
                                            ALU.add)
                    nc.vector.tensor_tensor(scores[:], scores[:], rsn_rep,
                                            ALU.mult)
                    nc.vector.tensor_tensor(scores[:], scores[:], rhn_v,
                                            ALU.mult)

                    if stage == "scores":
                        continue

                    # exp(scores): cosine sims are in [-1,1], no
                    # max-subtraction needed; normalizer rides the wsum
                    nc.scalar.activation(
                        out=exp_sc[:, h * HB:(h + 1) * HB, :], in_=scores[:],
                        func=AF.Exp)

                    # scatter exp(s) into the block-diagonal stationary
                    abd_flat = abd.rearrange("p t b -> p (t b)")
                    stride = 2 * G + 1
                    for t2 in range(2):
                        s0 = h * HB * stride + t2 * G
                        nc.vector.tensor_copy(
                            out=abd_flat[:, s0:s0 + (HB - 1) * stride
                                         + 1:stride],
                            in_=exp_sc[:, h * HB:(h + 1) * HB, t2])

                    pending.append(gh)
                    _pend = _envint('KB_PEND', 3)
                    if gh >= 2 * NG - _envint('KB_PENDTAIL', 0):
                        _pend = 1
                    while len(pending) > _pend:
                        emit_tail(pending.pop(0))

                while pending:
                    emit_tail(pending.pop(0))

    nc.compile()
    return nc


_NC = None


def _get_nc():
    global _NC
    if _NC is None:
        _NC = build_nc()
    return _NC


def _prep_core_inputs(state_emb, hints_emb, Ws, bs, Wh, bh, core):
    bf16 = ml_dtypes.bfloat16
    s = slice(core * BL, (core + 1) * BL)
    hf = np.ascontiguousarray(hints_emb[s]).reshape(BL * N, HD)
    hfb = hf.astype(bf16)
    hf8 = hf.astype(ml_dtypes.float8_e4m3)
    # natural: (g, p, t, f) with row = g*2048 + t*128 + p
    xnat = np.ascontiguousarray(
        hfb.reshape(NG, TPG, 128, 512).transpose(0, 2, 1, 3))
    # transposed: (g, p, t, k, r) with row = g*2048 + t*128 + r, h = k*128+p
    xtd = np.ascontiguousarray(
        hf8.reshape(NG, TPG, 128, KH, 128).transpose(0, 4, 1, 3, 2))
    bf = ml_dtypes.bfloat16
    # Ws.T arranged [s_in_chunk, s_chunk, a]
    wst = Ws.T.reshape(SD // 128, 128, AD).transpose(1, 0, 2)
    wh2 = Wh.reshape(2, 128, HD).transpose(1, 0, 2)
    # Wh.T arranged [h_in_chunk, h_chunk, a]
    wht = Wh.T.reshape(KH, 128, AD).transpose(1, 0, 2)
    bh2 = bh.reshape(2, 128).T
    bsb = np.ascontiguousarray(
        np.broadcast_to(bs, (BL, AD))).astype(np.float32)
    bhb = np.ascontiguousarray(
        np.broadcast_to(bh, (BL, AD))).astype(np.float32)
    st = np.asarray(state_emb[s])
    # state.T arranged [s_in_chunk, s_chunk, b]
    statet = st.T.reshape(SD // 128, 128, BL).transpose(1, 0, 2)
    wpack = np.concatenate([
        statet.reshape(128, -1), wst.reshape(128, -1),
        wh2.reshape(128, -1), wht.reshape(128, -1),
        bh2.reshape(128, -1),
    ], axis=1).astype(bf)
    wpack = np.ascontiguousarray(wpack)
    return {
        "wpack": wpack,
        "xnat": xnat,
        "xt": xtd,
        "bsb": bsb,
        "bhb": bhb,
    }


def kernel(state_emb, hints_emb, Ws, bs, Wh, bh):
    state_emb = np.asarray(state_emb, dtype=np.float32)
    hints_emb = np.asarray(hints_emb, dtype=np.float32)
    Ws = np.asarray(Ws, dtype=np.float32)
    bs = np.asarray(bs, dtype=np.float32)
    Wh = np.asarray(Wh, dtype=np.float32)
    bh = np.asarray(bh, dtype=np.float32)

    nc = _get_nc()
    in_maps = [
        _prep_core_inputs(state_emb, hints_emb, Ws, bs, Wh, bh, c)
        for c in range(NCORES)
    ]
    res = run_bass_kernel_spmd(nc, in_maps, core_ids=list(range(NCORES)))
    return np.concatenate([res.results[c]["out"] for c in range(NCORES)],
                          axis=0)

